# revision 6
# baseline (speedup 1.0000x reference)
"""Trainium2 Bass kernel for the temporal-shift multi-head attention module.

Sharding: data-parallel over the video axis — 8 videos of 8 frames each,
one video (8 frames x 197 tokens) per NeuronCore. The temporal head shift
only moves data between frames of the same video, so it is a pure slicing
operation on-device. Weights are replicated.

Per-core pipeline (all on-chip, bf16 matmul operands, fp32 accumulation):
  1. DMA x naturally, cast bf16, PE-transpose to xT [C(part), M(free)].
  2. Adapter: hT = aw1^T @ xT (+b1); xT += aw2^T @ hT (+b2)  (in-place x1).
  3. qT/kT = W^T @ x1T (channel-major); v = x1T^T @ Wv (token-major,
     stored per frame as [tok, head, 65] with a ones column appended).
  4. Attention per (frame, head): scoresT = k^T.T @ qT (keys on partitions),
     e = exp(scale*scoresT) on ACT, outT_aug = v_aug^T @ e via PE — row 64
     of outT_aug is the softmax denominator (ones column trick). Normalize
     with a K=2 broadcast matmul of the reciprocal rows.
  5. proj: out = aoT^T @ Wp + b, DMA out naturally.
"""

import numpy as np

F = 8
N = 197
C = 768
HADP = 192
NH = 12
HD = 64
M = F * N  # 1576
SCALE = HD ** -0.5
NCORES = 8
MCHUNKS = [(0, 512), (512, 512), (1024, 512), (1536, 40)]
MTILES = [(i * 128, 128) for i in range(12)] + [(1536, 40)]
JTILES = [(0, 128), (128, 69)]

_CACHE = {}


def _build():
    import concourse.mybir as mybir
    from concourse import bacc
    import concourse.tile as tile
    from concourse.masks import make_identity

    BF = mybir.dt.bfloat16
    FP = mybir.dt.float32
    AT = mybir.ActivationFunctionType
    OP = mybir.AluOpType

    nc = bacc.Bacc("TRN2", target_bir_lowering=False, debug=False)

    x_e = nc.dram_tensor("x", [F, N, C], FP, kind="ExternalInput")
    aw1_e = nc.dram_tensor("a_w1", [C, HADP], FP, kind="ExternalInput")
    ab1_e = nc.dram_tensor("a_b1", [HADP], FP, kind="ExternalInput")
    aw2_e = nc.dram_tensor("a_w2", [HADP, C], FP, kind="ExternalInput")
    ab2_e = nc.dram_tensor("a_b2", [C], FP, kind="ExternalInput")
    qkvw_e = nc.dram_tensor("qkv_w", [C, 3 * C], FP, kind="ExternalInput")
    qkvb_e = nc.dram_tensor("qkv_b", [3 * C], FP, kind="ExternalInput")
    projw_e = nc.dram_tensor("proj_w", [C, C], FP, kind="ExternalInput")
    projb_e = nc.dram_tensor("proj_b", [C], FP, kind="ExternalInput")
    out_e = nc.dram_tensor("out", [F, N, C], FP, kind="ExternalOutput")

    xf = x_e.rearrange("f n c -> (f n) c")
    outf = out_e.rearrange("f n c -> (f n) c")

    with tile.TileContext(nc) as tc:
        with tc.tile_pool(name="persist", bufs=1) as pp, \
             tc.tile_pool(name="scratch", bufs=2) as sp:
            # ---- constants
            ident = pp.tile([128, 128], BF, name="ident", tag="ident")
            make_identity(nc, ident)
            ones1 = pp.tile([1, 64], BF, name="ones1", tag="ones1")
            nc.vector.memset(ones1[:, :], 1.0)

            # ---- weights: DMA fp32 staging -> bf16 resident (gpsimd cast)
            def load_cast(name, p, fdim, src_ap):
                dst = pp.tile([p, fdim], BF, name=name, tag=name)
                stg = sp.tile([128, 2304], FP, name=f"stg_{name}", tag="wstage")
                nc.sync.dma_start(stg[0:p, 0:fdim], src_ap)
                nc.gpsimd.tensor_copy(dst[:, :], stg[0:p, 0:fdim])
                return dst

            qkvw = [load_cast(f"qkvw{k}", 128, 3 * C, qkvw_e[k * 128:(k + 1) * 128, :])
                    for k in range(6)]
            projw = [load_cast(f"projw{k}", 128, C, projw_e[k * 128:(k + 1) * 128, :])
                     for k in range(6)]
            aw1 = [load_cast(f"aw1_{k}", 128, HADP, aw1_e[k * 128:(k + 1) * 128, :])
                   for k in range(6)]
            aw2 = [load_cast("aw2_0", 128, C, aw2_e[0:128, :]),
                   load_cast("aw2_1", 64, C, aw2_e[128:HADP, :])]

            def load_col(name, p, src_ap):
                t = pp.tile([p, 1], FP, name=name, tag=name)
                nc.sync.dma_start(t[:, :], src_ap[:, None])
                return t

            b1c = [load_col("b1c0", 128, ab1_e[0:128]),
                   load_col("b1c1", 64, ab1_e[128:HADP])]
            b2c = [load_col(f"b2c{i}", 128, ab2_e[i * 128:(i + 1) * 128])
                   for i in range(6)]
            qkbc = [load_col(f"qkbc{i}", 128, qkvb_e[i * 128:(i + 1) * 128])
                    for i in range(12)]

            def load_bcast(name, src_ap):
                t = pp.tile([128, C], FP, name=name, tag=name)
                nc.sync.dma_start(t[:, :], src_ap[None, :].broadcast_to((128, C)))
                return t

            vbb = load_bcast("vbb", qkvb_e[2 * C:3 * C])
            pbb = load_bcast("pbb", projb_e[:])

            # ---- persistent activations
            xT = [pp.tile([128, M], BF, name=f"xT{i}", tag=f"xT{i}") for i in range(6)]
            hT = [pp.tile([128, M], BF, name="hT0", tag="hT0"),
                  pp.tile([64, M], BF, name="hT1", tag="hT1")]
            qT = [pp.tile([128, M], BF, name=f"qT{i}", tag=f"qT{i}") for i in range(6)]
            kT = [pp.tile([128, M], BF, name=f"kT{i}", tag=f"kT{i}") for i in range(6)]
            aoT = [pp.tile([128, M], BF, name=f"aoT{i}", tag=f"aoT{i}") for i in range(6)]
            vt = [[pp.tile([128, NH, HD + 1], BF, name=f"v{f}_{j}", tag=f"v{f}_{j}")
                   for j in range(2)] for f in range(F)]
            for f in range(F):
                for j in range(2):
                    nc.vector.memset(vt[f][j][:, :, HD:HD + 1], 1.0)

            # ---- phase 1: load x, cast, PE-transpose into xT
            with tc.tile_pool(name="pst", bufs=4, space="PSUM") as pst:
                for mt, (mb, msz) in enumerate(MTILES):
                    xn = sp.tile([128, C], FP, name=f"xn{mt}", tag="xn")
                    nc.sync.dma_start(xn[0:msz, :], xf[mb:mb + msz, :])
                    xb = sp.tile([128, C], BF, name=f"xb{mt}", tag="xb")
                    nc.gpsimd.tensor_copy(xb[0:msz, :], xn[0:msz, :])
                    for ct in range(6):
                        pt = pst.tile([128, 128], BF, name=f"pt{mt}_{ct}", tag="pt")
                        nc.tensor.transpose(pt[:, 0:msz],
                                            xb[0:msz, ct * 128:(ct + 1) * 128],
                                            ident[0:msz, 0:msz])
                        nc.scalar.activation(xT[ct][:, mb:mb + msz], pt[:, 0:msz],
                                             AT.Copy)

            # ---- phase 2+3: adapter (hT, then x1T in place into xT)
            with tc.tile_pool(name="psA", bufs=4, space="PSUM") as psA:
                for ht, (hb, hsz) in enumerate([(0, 128), (128, 64)]):
                    for mb, msz in MCHUNKS:
                        ps = psA.tile([128, 512], FP, name=f"psh{ht}_{mb}", tag="psA")
                        for kt in range(6):
                            nc.tensor.matmul(ps[0:hsz, 0:msz],
                                             aw1[kt][:, hb:hb + hsz],
                                             xT[kt][:, mb:mb + msz],
                                             start=(kt == 0), stop=(kt == 5))
                        nc.scalar.activation(hT[ht][:, mb:mb + msz], ps[0:hsz, 0:msz],
                                             AT.Identity, bias=b1c[ht][:, :])
                for ct in range(6):
                    for mb, msz in MCHUNKS:
                        ps = psA.tile([128, 512], FP, name=f"psx{ct}_{mb}", tag="psA")
                        for kt, ksz in enumerate([128, 64]):
                            nc.tensor.matmul(ps[:, 0:msz],
                                             aw2[kt][0:ksz, ct * 128:(ct + 1) * 128],
                                             hT[kt][0:ksz, mb:mb + msz],
                                             start=(kt == 0), stop=(kt == 1))
                        nc.vector.scalar_tensor_tensor(
                            out=xT[ct][:, mb:mb + msz], in0=ps[:, 0:msz],
                            scalar=b2c[ct][:, :], in1=xT[ct][:, mb:mb + msz],
                            op0=OP.add, op1=OP.add)

                # ---- phase 4a: qT / kT (channel-major)
                for ot in range(12):
                    dst = qT[ot] if ot < 6 else kT[ot - 6]
                    for mb, msz in MCHUNKS:
                        ps = psA.tile([128, 512], FP, name=f"psqk{ot}_{mb}", tag="psA")
                        for kt in range(6):
                            nc.tensor.matmul(ps[:, 0:msz],
                                             qkvw[kt][:, ot * 128:(ot + 1) * 128],
                                             xT[kt][:, mb:mb + msz],
                                             start=(kt == 0), stop=(kt == 5))
                        nc.scalar.activation(dst[:, mb:mb + msz], ps[:, 0:msz],
                                             AT.Identity, bias=qkbc[ot][:, :])

                # ---- phase 4b: v (token-major, per frame, ones col appended)
                for f in range(F):
                    for jt, (jb, jsz) in enumerate(JTILES):
                        for half in range(2):
                            ps = psA.tile([128, 512], FP,
                                          name=f"psv{f}_{jt}_{half}", tag="psA")
                            for kt in range(6):
                                nc.tensor.matmul(
                                    ps[0:jsz, 0:384],
                                    xT[kt][:, f * N + jb: f * N + jb + jsz],
                                    qkvw[kt][:, 1536 + half * 384: 1536 + (half + 1) * 384],
                                    start=(kt == 0), stop=(kt == 5))
                            nc.vector.tensor_tensor(
                                out=vt[f][jt][0:jsz, half * 6:(half + 1) * 6, 0:HD],
                                in0=ps[0:jsz, 0:384].rearrange("p (h d) -> p h d", d=HD),
                                in1=vbb[0:jsz, half * 384:(half + 1) * 384].rearrange(
                                    "p (h d) -> p h d", d=HD),
                                op=OP.add)

            # ---- phase 5: attention per (frame, head-pair)
            with tc.tile_pool(name="psT", bufs=1, space="PSUM") as psT:
                for f in range(F):
                    for hp in range(6):
                        if hp == 0:
                            fk = max(f - 1, 0)
                        elif hp == 1:
                            fk = min(f + 1, F - 1)
                        else:
                            fk = f
                        es = {}
                        for hi in range(2):
                            pb = hi * 64
                            for jt, (jb, jsz) in enumerate(JTILES):
                                ps = psT.tile([128, N], FP, bufs=4,
                                              name=f"st{f}_{hp}_{hi}_{jt}", tag="st")
                                nc.tensor.matmul(
                                    ps[0:jsz, :],
                                    kT[hp][pb:pb + 64, fk * N + jb: fk * N + jb + jsz],
                                    qT[hp][pb:pb + 64, f * N:(f + 1) * N],
                                    start=True, stop=True)
                                e = sp.tile([128, N], BF, bufs=8,
                                            name=f"e{f}_{hp}_{hi}_{jt}", tag="e")
                                nc.scalar.activation(e[0:jsz, :], ps[0:jsz, :],
                                                     AT.Exp, scale=SCALE)
                                es[(hi, jt)] = e
                        avs = []
                        drs = []
                        for hi in range(2):
                            av = psT.tile([HD + 1, N], FP, bufs=2,
                                          name=f"av{f}_{hp}_{hi}", tag="av")
                            for jt, (jb, jsz) in enumerate(JTILES):
                                nc.tensor.matmul(av[:, :],
                                                 vt[fk][jt][0:jsz, 2 * hp + hi, :],
                                                 es[(hi, jt)][0:jsz, :],
                                                 start=(jt == 0), stop=(jt == 1))
                            dr = sp.tile([1, N], BF, bufs=4,
                                         name=f"dr{f}_{hp}_{hi}", tag=f"dr{hi}")
                            with nc.allow_low_precision(reason="bf16 denom recip"):
                                nc.vector.reciprocal(dr[:, :], av[HD:HD + 1, :])
                            avs.append(av)
                            drs.append(dr)
                        for hi in range(2):
                            bc = psT.tile([64, N], FP, bufs=2,
                                          name=f"bc{f}_{hp}_{hi}", tag="bc")
                            nc.tensor.matmul(bc[:, :], ones1[:, :], drs[hi][:, :],
                                             start=True, stop=True)
                            bcs = sp.tile([64, N], BF, bufs=4,
                                          name=f"bcs{f}_{hp}_{hi}", tag="bcs")
                            nc.vector.tensor_copy(bcs[:, :], bc[:, :])
                            nc.vector.tensor_tensor(
                                out=aoT[hp][hi * 64:(hi + 1) * 64, f * N:(f + 1) * N],
                                in0=avs[hi][0:HD, :],
                                in1=bcs[:, :],
                                op=OP.mult)

            # ---- phase 6: proj + bias, DMA out
            with tc.tile_pool(name="psP", bufs=4, space="PSUM") as psP:
                for mt, (mb, msz) in enumerate(MTILES):
                    osb = sp.tile([128, C], FP, bufs=3, name=f"osb{mt}", tag="osb")
                    for half in range(2):
                        ps = psP.tile([128, 384], FP,
                                      name=f"psp{mt}_{half}", tag="psP")
                        for kt in range(6):
                            nc.tensor.matmul(ps[0:msz, :],
                                             aoT[kt][:, mb:mb + msz],
                                             projw[kt][:, half * 384:(half + 1) * 384],
                                             start=(kt == 0), stop=(kt == 5))
                        nc.vector.tensor_tensor(
                            out=osb[0:msz, half * 384:(half + 1) * 384],
                            in0=ps[0:msz, :],
                            in1=pbb[0:msz, half * 384:(half + 1) * 384],
                            op=OP.add)
                    nc.sync.dma_start(outf[mb:mb + msz, :], osb[0:msz, :])

    nc.compile()
    return nc


def _get_nc():
    if "nc" not in _CACHE:
        _CACHE["nc"] = _build()
    return _CACHE["nc"]


def _in_maps(inputs):
    x = np.ascontiguousarray(np.asarray(inputs["x"], np.float32))
    w = {k: np.ascontiguousarray(np.asarray(inputs[k], np.float32))
         for k in ("a_w1", "a_b1", "a_w2", "a_b2", "qkv_w", "qkv_b",
                   "proj_w", "proj_b")}
    maps = []
    for i in range(NCORES):
        m = {"x": x[i * F:(i + 1) * F]}
        m.update(w)
        maps.append(m)
    return maps


def kernel(**inputs):
    from concourse.bass_utils import run_bass_kernel_spmd
    nc = _get_nc()
    res = run_bass_kernel_spmd(nc, _in_maps(inputs), core_ids=list(range(NCORES)))
    return np.concatenate([res.results[i]["out"] for i in range(NCORES)], axis=0)


def run_traced(inputs, **kwargs):
    """Test harness helper: run with NTFF profiling, return (output, results)."""
    from concourse.bass_utils import run_bass_kernel_spmd
    nc = _get_nc()
    res = run_bass_kernel_spmd(nc, _in_maps(inputs),
                               core_ids=list(range(NCORES)), trace=True, **kwargs)
    out = np.concatenate([res.results[i]["out"] for i in range(NCORES)], axis=0)
    return out, res


# revision 22
# speedup vs baseline: 1.2404x; 1.2404x over previous
"""Trainium2 Bass kernel for the temporal-shift multi-head attention module.

Sharding: data-parallel over the video axis — 8 videos of 8 frames each,
one video (8 frames x 197 tokens) per NeuronCore. The temporal head shift
only moves data between frames of the same video, so it is a pure slicing
operation on-device. Weights are replicated.

Per-core pipeline (all on-chip, bf16 matmul operands, fp32 accumulation):
  1. DMA x naturally, cast bf16 (ACT), PE-transpose to xT [C(part), M(free)].
  2. Adapter: hT = aw1^T @ xT (+b1); xT += aw2^T @ hT (+b2)  (in-place x1).
  3. qT/kT = W^T @ x1T (channel-major); v = x1T^T @ Wv (token-major, stored
     per frame as [tok, head, 128]: 64 v-channels + 64 ones columns).
  4. Attention per (frame, head): scoresT = k^T.T @ qT (keys on partitions),
     e = exp(scale*scoresT) on ACT, av = [v_h | ones]^T @ e via PE — rows
     64:128 are the softmax denominator replicated 64x. Normalize with
     reciprocal_approx_fast + one DVE multiply into aoT.
  5. proj: out = aoT^T @ Wp + b, DMA out naturally.
"""

import numpy as np

F = 8
N = 197
C = 768
HADP = 192
NH = 12
HD = 64
M = F * N  # 1576
SCALE = HD ** -0.5
NCORES = 8
MCHUNKS = [(0, 512), (512, 512), (1024, 512), (1536, 40)]
MTILES = [(i * 128, 128) for i in range(12)] + [(1536, 40)]
JTILES = [(0, 128), (128, 69)]

_CACHE = {}


def _build():
    import concourse.mybir as mybir
    from concourse import bacc
    import concourse.tile as tile
    from concourse.masks import make_identity

    BF = mybir.dt.bfloat16
    FP = mybir.dt.float32
    AT = mybir.ActivationFunctionType
    OP = mybir.AluOpType

    nc = bacc.Bacc("TRN2", target_bir_lowering=False, debug=False)

    x_e = nc.dram_tensor("x", [F, N, C], FP, kind="ExternalInput")
    aw1_e = nc.dram_tensor("a_w1", [C, HADP], FP, kind="ExternalInput")
    ab1_e = nc.dram_tensor("a_b1", [HADP], FP, kind="ExternalInput")
    aw2_e = nc.dram_tensor("a_w2", [HADP, C], FP, kind="ExternalInput")
    ab2_e = nc.dram_tensor("a_b2", [C], FP, kind="ExternalInput")
    qkvw_e = nc.dram_tensor("qkv_w", [C, 3 * C], FP, kind="ExternalInput")
    qkvb_e = nc.dram_tensor("qkv_b", [3 * C], FP, kind="ExternalInput")
    projw_e = nc.dram_tensor("proj_w", [C, C], FP, kind="ExternalInput")
    projb_e = nc.dram_tensor("proj_b", [C], FP, kind="ExternalInput")
    out_e = nc.dram_tensor("out", [F, N, C], FP, kind="ExternalOutput")

    xf = x_e.rearrange("f n c -> (f n) c")
    outf = out_e.rearrange("f n c -> (f n) c")

    with tile.TileContext(nc) as tc:
        with tc.tile_pool(name="persist", bufs=1) as pp, \
             tc.tile_pool(name="scratch", bufs=2) as sp:
            # ---- constants
            ident = pp.tile([128, 128], BF, name="ident", tag="ident")
            make_identity(nc, ident)

            # ---- persistent activations
            xT = [pp.tile([128, M], BF, name=f"xT{i}", tag=f"xT{i}") for i in range(6)]
            qT = [pp.tile([128, M], BF, name=f"qT{i}", tag=f"qT{i}") for i in range(6)]
            kT = [pp.tile([128, M], BF, name=f"kT{i}", tag=f"kT{i}") for i in range(6)]
            aoT = [pp.tile([128, M], BF, name=f"aoT{i}", tag=f"aoT{i}") for i in range(6)]
            # v per frame/token-tile, natural token-major layout [tok, chan]
            vt = [[pp.tile([128, C], BF, name=f"v{f}_{j}", tag=f"v{f}_{j}")
                   for j in range(2)] for f in range(F)]
            # one-hot column-selector blocks: head h's denominator matmul
            # (lhsT = oneblock[:, h*12:(h+1)*12]) accumulates into row h of a
            # shared [12,197] PSUM tile.
            oneblock = pp.tile([128, NH * NH], BF, name="oneblock", tag="oneblock")
            nc.vector.memset(oneblock[:, :], 0.0)
            for h in range(NH):
                nc.vector.memset(oneblock[:, h * NH + h:h * NH + h + 1], 1.0)

            # ---- phase 1: load x, cast bf16 (ACT), PE-transpose into xT
            with tc.tile_pool(name="xload", bufs=2) as xp, \
                 tc.tile_pool(name="pst", bufs=4, space="PSUM") as pst:
                for mt, (mb, msz) in enumerate(MTILES):
                    xn = xp.tile([128, C], FP, name=f"xn{mt}", tag="xn")
                    nc.sync.dma_start(xn[0:msz, :], xf[mb:mb + msz, :])
                    xb = xp.tile([128, C], BF, name=f"xb{mt}", tag="xb")
                    nc.scalar.copy(xb[0:msz, :], xn[0:msz, :])
                    for ct in range(6):
                        pt = pst.tile([128, 128], BF, name=f"pt{mt}_{ct}", tag="pt")
                        nc.tensor.transpose(pt[:, 0:msz],
                                            xb[0:msz, ct * 128:(ct + 1) * 128],
                                            ident[0:msz, 0:msz])
                        nc.scalar.activation(xT[ct][:, mb:mb + msz], pt[:, 0:msz],
                                             AT.Copy)

            # ---- weights: chunked DMA fp32 staging -> bf16 resident (DVE cast)
            def load_cast(wp2, name, p, fdim, src_ap):
                dst = pp.tile([p, fdim], BF, name=name, tag=name)
                for cb in range(0, fdim, C):
                    csz = min(C, fdim - cb)
                    stg = wp2.tile([128, C], FP, name=f"stg_{name}_{cb}", tag="wstg")
                    nc.sync.dma_start(stg[0:p, 0:csz], src_ap[:, cb:cb + csz])
                    nc.vector.tensor_copy(dst[:, cb:cb + csz], stg[0:p, 0:csz])
                return dst

            def load_col(name, p, src_ap):
                t = pp.tile([p, 1], FP, name=name, tag=name)
                nc.sync.dma_start(t[:, :], src_ap[:, None])
                return t

            def load_bcast(name, src_ap):
                t = pp.tile([128, C], FP, name=name, tag=name)
                nc.sync.dma_start(t[:, :], src_ap[None, :].broadcast_to((128, C)))
                return t

            with tc.tile_pool(name="wstg", bufs=3) as wp2:
                aw1 = [load_cast(wp2, f"aw1_{k}", 128, HADP,
                                 aw1_e[k * 128:(k + 1) * 128, :]) for k in range(6)]
                aw2 = [load_cast(wp2, "aw2_0", 128, C, aw2_e[0:128, :]),
                       load_cast(wp2, "aw2_1", 64, C, aw2_e[128:HADP, :])]
                qkvw = [load_cast(wp2, f"qkvw{k}", 128, 3 * C,
                                  qkvw_e[k * 128:(k + 1) * 128, :]) for k in range(6)]
                projw = [load_cast(wp2, f"projw{k}", 128, C,
                                   projw_e[k * 128:(k + 1) * 128, :]) for k in range(6)]

            b1c = [load_col("b1c0", 128, ab1_e[0:128]),
                   load_col("b1c1", 64, ab1_e[128:HADP])]
            b2c = [load_col(f"b2c{i}", 128, ab2_e[i * 128:(i + 1) * 128])
                   for i in range(6)]
            qkbc = [load_col(f"qkbc{i}", 128, qkvb_e[i * 128:(i + 1) * 128])
                    for i in range(12)]
            vbb = load_bcast("vbb", qkvb_e[2 * C:3 * C])
            pbb = load_bcast("pbb", projb_e[:])

            # ---- phase 2+3: adapter (hT, then x1T in place into xT)
            with tc.tile_pool(name="hpool", bufs=1) as hp, \
                 tc.tile_pool(name="psA", bufs=4, space="PSUM") as psA:
                hT = [hp.tile([128, M], BF, name="hT0", tag="hT0"),
                      hp.tile([64, M], BF, name="hT1", tag="hT1")]
                for ht, (hb, hsz) in enumerate([(0, 128), (128, 64)]):
                    for mb, msz in MCHUNKS:
                        ps = psA.tile([128, 512], FP, name=f"psh{ht}_{mb}", tag="psA")
                        for kt in range(6):
                            nc.tensor.matmul(ps[0:hsz, 0:msz],
                                             aw1[kt][:, hb:hb + hsz],
                                             xT[kt][:, mb:mb + msz],
                                             start=(kt == 0), stop=(kt == 5))
                        nc.scalar.activation(hT[ht][:, mb:mb + msz], ps[0:hsz, 0:msz],
                                             AT.Identity, bias=b1c[ht][:, :])
                for ct in range(6):
                    for mb, msz in MCHUNKS:
                        ps = psA.tile([128, 512], FP, name=f"psx{ct}_{mb}", tag="psA")
                        for kt, ksz in enumerate([128, 64]):
                            nc.tensor.matmul(ps[:, 0:msz],
                                             aw2[kt][0:ksz, ct * 128:(ct + 1) * 128],
                                             hT[kt][0:ksz, mb:mb + msz],
                                             start=(kt == 0), stop=(kt == 1))
                        nc.vector.scalar_tensor_tensor(
                            out=xT[ct][:, mb:mb + msz], in0=ps[:, 0:msz],
                            scalar=b2c[ct][:, :], in1=xT[ct][:, mb:mb + msz],
                            op0=OP.add, op1=OP.add)

            with tc.tile_pool(name="psB", bufs=4, space="PSUM") as psB:
                # ---- phase 4a: qT / kT (channel-major)
                for ot in range(12):
                    dst = qT[ot] if ot < 6 else kT[ot - 6]
                    for mb, msz in MCHUNKS:
                        ps = psB.tile([128, 512], FP, name=f"psqk{ot}_{mb}", tag="psB")
                        for kt in range(6):
                            nc.tensor.matmul(ps[:, 0:msz],
                                             qkvw[kt][:, ot * 128:(ot + 1) * 128],
                                             xT[kt][:, mb:mb + msz],
                                             start=(kt == 0), stop=(kt == 5))
                        nc.scalar.activation(dst[:, mb:mb + msz], ps[:, 0:msz],
                                             AT.Identity, bias=qkbc[ot][:, :])

                # ---- phase 4b: v (token-major, per frame, interleaved ones)
                for f in range(F):
                    for jt, (jb, jsz) in enumerate(JTILES):
                        for half in range(2):
                            ps = psB.tile([128, 512], FP,
                                          name=f"psv{f}_{jt}_{half}", tag="psB")
                            for kt in range(6):
                                nc.tensor.matmul(
                                    ps[0:jsz, 0:384],
                                    xT[kt][:, f * N + jb: f * N + jb + jsz],
                                    qkvw[kt][:, 1536 + half * 384: 1536 + (half + 1) * 384],
                                    start=(kt == 0), stop=(kt == 5))
                            nc.vector.tensor_tensor(
                                out=vt[f][jt][0:jsz, half * 384:(half + 1) * 384],
                                in0=ps[0:jsz, 0:384],
                                in1=vbb[0:jsz, half * 384:(half + 1) * 384],
                                op=OP.add)

            # ---- phase 5: attention, per frame
            # Scores/exps as in the per-head-pair scheme (full-bank PSUM tiles,
            # row-group packing). Denominators: 24 accumulating one-hot
            # matmuls into one [12,197] bank, ONE batched DVE reciprocal per
            # frame (reciprocal cost is free-size-bound), DMA bounce through
            # DRAM to row-broadcast, then av-matmul + one DVE mult per head.
            def fk_of(f, h):
                if h < 2:
                    return max(f - 1, 0)
                if h < 4:
                    return min(f + 1, F - 1)
                return f

            with tc.tile_pool(name="psT", bufs=1, space="PSUM") as psT, \
                 tc.tile_pool(name="drp", bufs=2, space="DRAM") as drp:
                for f in range(F):
                    es = {}
                    for hp_ in range(6):
                        fk = fk_of(f, 2 * hp_)
                        for hi in range(2):
                            pb = hi * 64
                            for jt, (jb, jsz) in enumerate(JTILES):
                                ps = psT.tile([128, N], FP, bufs=4,
                                              name=f"st{f}_{hp_}_{hi}_{jt}", tag="st")
                                nc.tensor.matmul(
                                    ps[0:jsz, :],
                                    kT[hp_][pb:pb + 64, fk * N + jb: fk * N + jb + jsz],
                                    qT[hp_][pb:pb + 64, f * N:(f + 1) * N],
                                    start=True, stop=True)
                                e = sp.tile([128, N], BF, bufs=26,
                                            name=f"e{f}_{hp_}_{hi}_{jt}", tag="e")
                                nc.scalar.activation(e[0:jsz, :], ps[0:jsz, :],
                                                     AT.Exp, scale=SCALE)
                                es[(2 * hp_ + hi, jt)] = e
                    den12 = psT.tile([NH, N], FP, bufs=2,
                                     name=f"den{f}", tag="den")
                    nmm = 0
                    for h in range(NH):
                        for jt, (jb, jsz) in enumerate(JTILES):
                            nc.tensor.matmul(
                                den12[:, :],
                                oneblock[0:jsz, h * NH:(h + 1) * NH],
                                es[(h, jt)][0:jsz, :],
                                start=(nmm == 0), stop=(nmm == 2 * NH - 1))
                            nmm += 1
                    rec12 = sp.tile([NH, N], FP, bufs=2, name=f"rcp{f}",
                                    tag="rec12")
                    nc.vector.reciprocal(rec12[:, :], den12[:, :])
                    dr12 = drp.tile([NH, N], FP, name=f"dr12_{f}", tag="dr12")
                    nc.sync.dma_start(dr12[:, :], rec12[:, :])
                    rec64s = []
                    for h in range(NH):
                        rec64 = sp.tile([64, N], FP, bufs=8,
                                        name=f"rec64_{f}_{h}", tag="rec64")
                        nc.sync.dma_start(rec64[:, :],
                                          dr12[h, :][None, :].broadcast_to((64, N)))
                        rec64s.append(rec64)
                    for h in range(NH):
                        fk = fk_of(f, h)
                        av = psT.tile([64, N], FP, bufs=2,
                                      name=f"av{f}_{h}", tag="av")
                        for jt, (jb, jsz) in enumerate(JTILES):
                            nc.tensor.matmul(av[:, :],
                                             vt[fk][jt][0:jsz, h * HD:(h + 1) * HD],
                                             es[(h, jt)][0:jsz, :],
                                             start=(jt == 0), stop=(jt == 1))
                        nc.vector.tensor_tensor(
                            out=aoT[h // 2][(h % 2) * 64:(h % 2 + 1) * 64,
                                            f * N:(f + 1) * N],
                            in0=av[:, :],
                            in1=rec64s[h][:, :],
                            op=OP.mult)

            # ---- phase 6: proj + bias, DMA out
            with tc.tile_pool(name="psP", bufs=4, space="PSUM") as psP:
                for mt, (mb, msz) in enumerate(MTILES):
                    osb = sp.tile([128, C], FP, bufs=3, name=f"osb{mt}", tag="osb")
                    for half in range(2):
                        ps = psP.tile([128, 384], FP,
                                      name=f"psp{mt}_{half}", tag="psP")
                        for kt in range(6):
                            nc.tensor.matmul(ps[0:msz, :],
                                             aoT[kt][:, mb:mb + msz],
                                             projw[kt][:, half * 384:(half + 1) * 384],
                                             start=(kt == 0), stop=(kt == 5))
                        nc.vector.tensor_tensor(
                            out=osb[0:msz, half * 384:(half + 1) * 384],
                            in0=ps[0:msz, :],
                            in1=pbb[0:msz, half * 384:(half + 1) * 384],
                            op=OP.add)
                    nc.sync.dma_start(outf[mb:mb + msz, :], osb[0:msz, :])

    nc.compile()
    return nc


def _get_nc():
    if "nc" not in _CACHE:
        _CACHE["nc"] = _build()
    return _CACHE["nc"]


def _in_maps(inputs):
    x = np.ascontiguousarray(np.asarray(inputs["x"], np.float32))
    w = {k: np.ascontiguousarray(np.asarray(inputs[k], np.float32))
         for k in ("a_w1", "a_b1", "a_w2", "a_b2", "qkv_w", "qkv_b",
                   "proj_w", "proj_b")}
    maps = []
    for i in range(NCORES):
        m = {"x": x[i * F:(i + 1) * F]}
        m.update(w)
        maps.append(m)
    return maps


def kernel(**inputs):
    from concourse.bass_utils import run_bass_kernel_spmd
    nc = _get_nc()
    res = run_bass_kernel_spmd(nc, _in_maps(inputs), core_ids=list(range(NCORES)))
    return np.concatenate([res.results[i]["out"] for i in range(NCORES)], axis=0)


def run_traced(inputs, **kwargs):
    """Test harness helper: run with NTFF profiling, return (output, results)."""
    from concourse.bass_utils import run_bass_kernel_spmd
    nc = _get_nc()
    res = run_bass_kernel_spmd(nc, _in_maps(inputs),
                               core_ids=list(range(NCORES)), trace=True, **kwargs)
    out = np.concatenate([res.results[i]["out"] for i in range(NCORES)], axis=0)
    return out, res


# revision 23
# speedup vs baseline: 1.3972x; 1.1264x over previous
"""Trainium2 Bass kernel for the temporal-shift multi-head attention module.

Sharding: data-parallel over the video axis — 8 videos of 8 frames each,
one video (8 frames x 197 tokens) per NeuronCore. The temporal head shift
only moves data between frames of the same video, so it is a pure slicing
operation on-device. Weights are replicated.

Per-core pipeline (all on-chip, bf16 matmul operands, fp32 accumulation):
  1. DMA x naturally, cast bf16 (ACT), PE-transpose to xT [C(part), M(free)].
  2. Adapter: hT = aw1^T @ xT (+b1); xT += aw2^T @ hT (+b2)  (in-place x1).
  3. qT/kT = W^T @ x1T (channel-major); v = x1T^T @ Wv (token-major, stored
     per frame as [tok, head, 128]: 64 v-channels + 64 ones columns).
  4. Attention per (frame, head): scoresT = k^T.T @ qT (keys on partitions),
     e = exp(scale*scoresT) on ACT, av = [v_h | ones]^T @ e via PE — rows
     64:128 are the softmax denominator replicated 64x. Normalize with
     reciprocal_approx_fast + one DVE multiply into aoT.
  5. proj: out = aoT^T @ Wp + b, DMA out naturally.
"""

import numpy as np

F = 8
N = 197
C = 768
HADP = 192
NH = 12
HD = 64
M = F * N  # 1576
SCALE = HD ** -0.5
NCORES = 8
MCHUNKS = [(0, 512), (512, 512), (1024, 512), (1536, 40)]
MTILES = [(i * 128, 128) for i in range(12)] + [(1536, 40)]
JTILES = [(0, 128), (128, 69)]

_CACHE = {}


def _build():
    import concourse.mybir as mybir
    from concourse import bacc
    import concourse.tile as tile
    from concourse.masks import make_identity

    BF = mybir.dt.bfloat16
    FP = mybir.dt.float32
    AT = mybir.ActivationFunctionType
    OP = mybir.AluOpType

    nc = bacc.Bacc("TRN2", target_bir_lowering=False, debug=False)

    x_e = nc.dram_tensor("x", [F, N, C], FP, kind="ExternalInput")
    aw1_e = nc.dram_tensor("a_w1", [C, HADP], FP, kind="ExternalInput")
    ab1_e = nc.dram_tensor("a_b1", [HADP], FP, kind="ExternalInput")
    aw2_e = nc.dram_tensor("a_w2", [HADP, C], FP, kind="ExternalInput")
    ab2_e = nc.dram_tensor("a_b2", [C], FP, kind="ExternalInput")
    qkvw_e = nc.dram_tensor("qkv_w", [C, 3 * C], FP, kind="ExternalInput")
    qkvb_e = nc.dram_tensor("qkv_b", [3 * C], FP, kind="ExternalInput")
    projw_e = nc.dram_tensor("proj_w", [C, C], FP, kind="ExternalInput")
    projb_e = nc.dram_tensor("proj_b", [C], FP, kind="ExternalInput")
    out_e = nc.dram_tensor("out", [F, N, C], FP, kind="ExternalOutput")

    xf = x_e.rearrange("f n c -> (f n) c")
    outf = out_e.rearrange("f n c -> (f n) c")

    with tile.TileContext(nc) as tc:
        with tc.tile_pool(name="persist", bufs=1) as pp, \
             tc.tile_pool(name="scratch", bufs=2) as sp:
            # ---- constants
            ident = pp.tile([128, 128], BF, name="ident", tag="ident")
            make_identity(nc, ident)

            # ---- persistent activations
            xT = [pp.tile([128, M], BF, name=f"xT{i}", tag=f"xT{i}") for i in range(6)]
            qT = [pp.tile([128, M], BF, name=f"qT{i}", tag=f"qT{i}") for i in range(6)]
            kT = [pp.tile([128, M], BF, name=f"kT{i}", tag=f"kT{i}") for i in range(6)]
            aoT = [pp.tile([128, M], BF, name=f"aoT{i}", tag=f"aoT{i}") for i in range(6)]
            # v per frame/token-tile, natural token-major layout [tok, chan]
            vt = [[pp.tile([128, C], BF, name=f"v{f}_{j}", tag=f"v{f}_{j}")
                   for j in range(2)] for f in range(F)]
            # one-hot column-selector blocks: head h's denominator matmul
            # (lhsT = oneblock[:, h*12:(h+1)*12]) accumulates into row h of a
            # shared [12,197] PSUM tile.
            oneblock = pp.tile([128, NH * NH], BF, name="oneblock", tag="oneblock")
            nc.vector.memset(oneblock[:, :], 0.0)
            for h in range(NH):
                nc.vector.memset(oneblock[:, h * NH + h:h * NH + h + 1], 1.0)

            # ---- phase 1: load x, cast bf16 (ACT), PE-transpose into xT
            # x staging shares slots with the weight staging (tag "wstg") so
            # no pool-release barrier separates the x pipeline from weight
            # DMAs — they stream through the same 4 slots back to back.
            with tc.tile_pool(name="pst", bufs=4, space="PSUM") as pst:
                for mt, (mb, msz) in enumerate(MTILES):
                    xn = sp.tile([128, C], FP, bufs=4, name=f"xn{mt}", tag="wstg")
                    nc.sync.dma_start(xn[0:msz, :], xf[mb:mb + msz, :])
                    xb = sp.tile([128, C], BF, bufs=3, name=f"xb{mt}", tag="xb")
                    nc.scalar.copy(xb[0:msz, :], xn[0:msz, :])
                    for ct in range(6):
                        pt = pst.tile([128, 128], BF, name=f"pt{mt}_{ct}", tag="pt")
                        nc.tensor.transpose(pt[:, 0:msz],
                                            xb[0:msz, ct * 128:(ct + 1) * 128],
                                            ident[0:msz, 0:msz])
                        nc.scalar.activation(xT[ct][:, mb:mb + msz], pt[:, 0:msz],
                                             AT.Copy)

            # ---- weights: chunked DMA fp32 staging -> bf16 resident (DVE cast)
            def load_cast(name, p, fdim, src_ap):
                dst = pp.tile([p, fdim], BF, name=name, tag=name)
                for cb in range(0, fdim, C):
                    csz = min(C, fdim - cb)
                    stg = sp.tile([128, C], FP, bufs=4,
                                  name=f"stg_{name}_{cb}", tag="wstg")
                    nc.sync.dma_start(stg[0:p, 0:csz], src_ap[:, cb:cb + csz])
                    nc.vector.tensor_copy(dst[:, cb:cb + csz], stg[0:p, 0:csz])
                return dst

            def load_col(name, p, src_ap):
                t = pp.tile([p, 1], FP, name=name, tag=name)
                nc.sync.dma_start(t[:, :], src_ap[:, None])
                return t

            def load_bcast(name, src_ap):
                t = pp.tile([128, C], FP, name=name, tag=name)
                nc.sync.dma_start(t[:, :], src_ap[None, :].broadcast_to((128, C)))
                return t

            aw1 = [load_cast(f"aw1_{k}", 128, HADP,
                             aw1_e[k * 128:(k + 1) * 128, :]) for k in range(6)]
            aw2 = [load_cast("aw2_0", 128, C, aw2_e[0:128, :]),
                   load_cast("aw2_1", 64, C, aw2_e[128:HADP, :])]
            qkvw = [load_cast(f"qkvw{k}", 128, 3 * C,
                              qkvw_e[k * 128:(k + 1) * 128, :]) for k in range(6)]
            projw = [load_cast(f"projw{k}", 128, C,
                               projw_e[k * 128:(k + 1) * 128, :]) for k in range(6)]

            b1c = [load_col("b1c0", 128, ab1_e[0:128]),
                   load_col("b1c1", 64, ab1_e[128:HADP])]
            b2c = [load_col(f"b2c{i}", 128, ab2_e[i * 128:(i + 1) * 128])
                   for i in range(6)]
            qkbc = [load_col(f"qkbc{i}", 128, qkvb_e[i * 128:(i + 1) * 128])
                    for i in range(12)]
            vbb = load_bcast("vbb", qkvb_e[2 * C:3 * C])
            pbb = load_bcast("pbb", projb_e[:])

            # ---- phase 2+3: adapter (hT, then x1T in place into xT)
            with tc.tile_pool(name="psA", bufs=4, space="PSUM") as psA:
                hT = [sp.tile([128, M], BF, bufs=1, name="hT0", tag="hT0"),
                      sp.tile([64, M], BF, bufs=1, name="hT1", tag="hT1")]
                for ht, (hb, hsz) in enumerate([(0, 128), (128, 64)]):
                    for mb, msz in MCHUNKS:
                        ps = psA.tile([128, 512], FP, name=f"psh{ht}_{mb}", tag="psA")
                        for kt in range(6):
                            nc.tensor.matmul(ps[0:hsz, 0:msz],
                                             aw1[kt][:, hb:hb + hsz],
                                             xT[kt][:, mb:mb + msz],
                                             start=(kt == 0), stop=(kt == 5))
                        nc.scalar.activation(hT[ht][:, mb:mb + msz], ps[0:hsz, 0:msz],
                                             AT.Identity, bias=b1c[ht][:, :])
                for ct in range(6):
                    for mb, msz in MCHUNKS:
                        ps = psA.tile([128, 512], FP, name=f"psx{ct}_{mb}", tag="psA")
                        for kt, ksz in enumerate([128, 64]):
                            nc.tensor.matmul(ps[:, 0:msz],
                                             aw2[kt][0:ksz, ct * 128:(ct + 1) * 128],
                                             hT[kt][0:ksz, mb:mb + msz],
                                             start=(kt == 0), stop=(kt == 1))
                        nc.vector.scalar_tensor_tensor(
                            out=xT[ct][:, mb:mb + msz], in0=ps[:, 0:msz],
                            scalar=b2c[ct][:, :], in1=xT[ct][:, mb:mb + msz],
                            op0=OP.add, op1=OP.add)

                # ---- phase 4a: qT / kT (channel-major)
                for ot in range(12):
                    dst = qT[ot] if ot < 6 else kT[ot - 6]
                    for mb, msz in MCHUNKS:
                        ps = psA.tile([128, 512], FP, name=f"psqk{ot}_{mb}", tag="psA")
                        for kt in range(6):
                            nc.tensor.matmul(ps[:, 0:msz],
                                             qkvw[kt][:, ot * 128:(ot + 1) * 128],
                                             xT[kt][:, mb:mb + msz],
                                             start=(kt == 0), stop=(kt == 5))
                        nc.scalar.activation(dst[:, mb:mb + msz], ps[:, 0:msz],
                                             AT.Identity, bias=qkbc[ot][:, :])

                # ---- phase 4b: v (token-major, per frame, interleaved ones)
                for f in range(F):
                    for jt, (jb, jsz) in enumerate(JTILES):
                        for half in range(2):
                            ps = psA.tile([128, 512], FP,
                                          name=f"psv{f}_{jt}_{half}", tag="psA")
                            for kt in range(6):
                                nc.tensor.matmul(
                                    ps[0:jsz, 0:384],
                                    xT[kt][:, f * N + jb: f * N + jb + jsz],
                                    qkvw[kt][:, 1536 + half * 384: 1536 + (half + 1) * 384],
                                    start=(kt == 0), stop=(kt == 5))
                            nc.vector.tensor_tensor(
                                out=vt[f][jt][0:jsz, half * 384:(half + 1) * 384],
                                in0=ps[0:jsz, 0:384],
                                in1=vbb[0:jsz, half * 384:(half + 1) * 384],
                                op=OP.add)

            # ---- phase 5: attention, per frame
            # Scores/exps as in the per-head-pair scheme (full-bank PSUM tiles,
            # row-group packing). Denominators: 24 accumulating one-hot
            # matmuls into one [12,197] bank, ONE batched DVE reciprocal per
            # frame (reciprocal cost is free-size-bound), DMA bounce through
            # DRAM to row-broadcast, then av-matmul + one DVE mult per head.
            def fk_of(f, h):
                if h < 2:
                    return max(f - 1, 0)
                if h < 4:
                    return min(f + 1, F - 1)
                return f

            with tc.tile_pool(name="psT", bufs=1, space="PSUM") as psT, \
                 tc.tile_pool(name="drp", bufs=2, space="DRAM") as drp:
                for f in range(F):
                    es = {}
                    for hp_ in range(6):
                        fk = fk_of(f, 2 * hp_)
                        for hi in range(2):
                            pb = hi * 64
                            for jt, (jb, jsz) in enumerate(JTILES):
                                ps = psT.tile([128, N], FP, bufs=4,
                                              name=f"st{f}_{hp_}_{hi}_{jt}", tag="st")
                                nc.tensor.matmul(
                                    ps[0:jsz, :],
                                    kT[hp_][pb:pb + 64, fk * N + jb: fk * N + jb + jsz],
                                    qT[hp_][pb:pb + 64, f * N:(f + 1) * N],
                                    start=True, stop=True)
                                e = sp.tile([128, N], BF, bufs=26,
                                            name=f"e{f}_{hp_}_{hi}_{jt}", tag="e")
                                nc.scalar.activation(e[0:jsz, :], ps[0:jsz, :],
                                                     AT.Exp, scale=SCALE)
                                es[(2 * hp_ + hi, jt)] = e
                    den12 = psT.tile([NH, N], FP, bufs=2,
                                     name=f"den{f}", tag="den")
                    nmm = 0
                    for h in range(NH):
                        for jt, (jb, jsz) in enumerate(JTILES):
                            nc.tensor.matmul(
                                den12[:, :],
                                oneblock[0:jsz, h * NH:(h + 1) * NH],
                                es[(h, jt)][0:jsz, :],
                                start=(nmm == 0), stop=(nmm == 2 * NH - 1))
                            nmm += 1
                    rec12 = sp.tile([NH, N], FP, bufs=2, name=f"rcp{f}",
                                    tag="rec12")
                    nc.vector.reciprocal(rec12[:, :], den12[:, :])
                    dr12 = drp.tile([NH, N], FP, name=f"dr12_{f}", tag="dr12")
                    nc.sync.dma_start(dr12[:, :], rec12[:, :])
                    rec64s = []
                    for h in range(NH):
                        rec64 = sp.tile([64, N], FP, bufs=6,
                                        name=f"rec64_{f}_{h}", tag="rec64")
                        nc.sync.dma_start(rec64[:, :],
                                          dr12[h, :][None, :].broadcast_to((64, N)))
                        rec64s.append(rec64)
                    for h in range(NH):
                        fk = fk_of(f, h)
                        av = psT.tile([64, N], FP, bufs=2,
                                      name=f"av{f}_{h}", tag="av")
                        for jt, (jb, jsz) in enumerate(JTILES):
                            nc.tensor.matmul(av[:, :],
                                             vt[fk][jt][0:jsz, h * HD:(h + 1) * HD],
                                             es[(h, jt)][0:jsz, :],
                                             start=(jt == 0), stop=(jt == 1))
                        nc.vector.tensor_tensor(
                            out=aoT[h // 2][(h % 2) * 64:(h % 2 + 1) * 64,
                                            f * N:(f + 1) * N],
                            in0=av[:, :],
                            in1=rec64s[h][:, :],
                            op=OP.mult)

            # ---- phase 6: proj + bias, DMA out
            with tc.tile_pool(name="psP", bufs=4, space="PSUM") as psP:
                for mt, (mb, msz) in enumerate(MTILES):
                    osb = sp.tile([128, C], FP, bufs=2, name=f"osb{mt}", tag="osb")
                    for half in range(2):
                        ps = psP.tile([128, 384], FP,
                                      name=f"psp{mt}_{half}", tag="psP")
                        for kt in range(6):
                            nc.tensor.matmul(ps[0:msz, :],
                                             aoT[kt][:, mb:mb + msz],
                                             projw[kt][:, half * 384:(half + 1) * 384],
                                             start=(kt == 0), stop=(kt == 5))
                        nc.vector.tensor_tensor(
                            out=osb[0:msz, half * 384:(half + 1) * 384],
                            in0=ps[0:msz, :],
                            in1=pbb[0:msz, half * 384:(half + 1) * 384],
                            op=OP.add)
                    nc.sync.dma_start(outf[mb:mb + msz, :], osb[0:msz, :])

    nc.compile()
    return nc


def _get_nc():
    if "nc" not in _CACHE:
        _CACHE["nc"] = _build()
    return _CACHE["nc"]


def _in_maps(inputs):
    x = np.ascontiguousarray(np.asarray(inputs["x"], np.float32))
    w = {k: np.ascontiguousarray(np.asarray(inputs[k], np.float32))
         for k in ("a_w1", "a_b1", "a_w2", "a_b2", "qkv_w", "qkv_b",
                   "proj_w", "proj_b")}
    maps = []
    for i in range(NCORES):
        m = {"x": x[i * F:(i + 1) * F]}
        m.update(w)
        maps.append(m)
    return maps


def kernel(**inputs):
    from concourse.bass_utils import run_bass_kernel_spmd
    nc = _get_nc()
    res = run_bass_kernel_spmd(nc, _in_maps(inputs), core_ids=list(range(NCORES)))
    return np.concatenate([res.results[i]["out"] for i in range(NCORES)], axis=0)


def run_traced(inputs, **kwargs):
    """Test harness helper: run with NTFF profiling, return (output, results)."""
    from concourse.bass_utils import run_bass_kernel_spmd
    nc = _get_nc()
    res = run_bass_kernel_spmd(nc, _in_maps(inputs),
                               core_ids=list(range(NCORES)), trace=True, **kwargs)
    out = np.concatenate([res.results[i]["out"] for i in range(NCORES)], axis=0)
    return out, res


# revision 24
# speedup vs baseline: 1.4716x; 1.0533x over previous
"""Trainium2 Bass kernel for the temporal-shift multi-head attention module.

Sharding: data-parallel over the video axis — 8 videos of 8 frames each,
one video (8 frames x 197 tokens) per NeuronCore. The temporal head shift
only moves data between frames of the same video, so it is a pure slicing
operation on-device. Weights are replicated.

Per-core pipeline (all on-chip, bf16 matmul operands, fp32 accumulation):
  1. DMA x naturally, cast bf16 (ACT), PE-transpose to xT [C(part), M(free)].
  2. Adapter: hT = aw1^T @ xT (+b1); xT += aw2^T @ hT (+b2)  (in-place x1).
  3. qT/kT = W^T @ x1T (channel-major); v = x1T^T @ Wv (token-major, stored
     per frame as [tok, head, 128]: 64 v-channels + 64 ones columns).
  4. Attention per (frame, head): scoresT = k^T.T @ qT (keys on partitions),
     e = exp(scale*scoresT) on ACT, av = [v_h | ones]^T @ e via PE — rows
     64:128 are the softmax denominator replicated 64x. Normalize with
     reciprocal_approx_fast + one DVE multiply into aoT.
  5. proj: out = aoT^T @ Wp + b, DMA out naturally.
"""

import numpy as np

F = 8
N = 197
C = 768
HADP = 192
NH = 12
HD = 64
M = F * N  # 1576
SCALE = HD ** -0.5
NCORES = 8
MCHUNKS = [(0, 512), (512, 512), (1024, 512), (1536, 40)]
MTILES = [(i * 128, 128) for i in range(12)] + [(1536, 40)]
JTILES = [(0, 128), (128, 69)]

_CACHE = {}


def _build():
    import concourse.mybir as mybir
    from concourse import bacc
    import concourse.tile as tile
    from concourse.masks import make_identity

    BF = mybir.dt.bfloat16
    FP = mybir.dt.float32
    AT = mybir.ActivationFunctionType
    OP = mybir.AluOpType

    nc = bacc.Bacc("TRN2", target_bir_lowering=False, debug=False)

    x_e = nc.dram_tensor("x", [F, N, C], FP, kind="ExternalInput")
    aw1_e = nc.dram_tensor("a_w1", [C, HADP], FP, kind="ExternalInput")
    ab1_e = nc.dram_tensor("a_b1", [HADP], FP, kind="ExternalInput")
    aw2_e = nc.dram_tensor("a_w2", [HADP, C], FP, kind="ExternalInput")
    ab2_e = nc.dram_tensor("a_b2", [C], FP, kind="ExternalInput")
    qkvw_e = nc.dram_tensor("qkv_w", [C, 3 * C], FP, kind="ExternalInput")
    qkvb_e = nc.dram_tensor("qkv_b", [3 * C], FP, kind="ExternalInput")
    projw_e = nc.dram_tensor("proj_w", [C, C], FP, kind="ExternalInput")
    projb_e = nc.dram_tensor("proj_b", [C], FP, kind="ExternalInput")
    out_e = nc.dram_tensor("out", [F, N, C], FP, kind="ExternalOutput")

    xf = x_e.rearrange("f n c -> (f n) c")
    outf = out_e.rearrange("f n c -> (f n) c")

    with tile.TileContext(nc) as tc:
        with tc.tile_pool(name="persist", bufs=1) as pp, \
             tc.tile_pool(name="scratch", bufs=2) as sp:
            # ---- constants
            ident = pp.tile([128, 128], BF, name="ident", tag="ident")
            make_identity(nc, ident)

            # ---- persistent activations
            xT = [pp.tile([128, M], BF, name=f"xT{i}", tag=f"xT{i}") for i in range(6)]
            qT = [pp.tile([128, M], BF, name=f"qT{i}", tag=f"qT{i}") for i in range(6)]
            kT = [pp.tile([128, M], BF, name=f"kT{i}", tag=f"kT{i}") for i in range(6)]
            aoT = [pp.tile([128, M], BF, name=f"aoT{i}", tag=f"aoT{i}") for i in range(6)]
            # v per frame/token-tile, natural token-major layout [tok, chan]
            vt = [[pp.tile([128, C], BF, name=f"v{f}_{j}", tag=f"v{f}_{j}")
                   for j in range(2)] for f in range(F)]
            # one-hot column-selector blocks: head h's denominator matmul
            # (lhsT = oneblock[:, h*12:(h+1)*12]) accumulates into row h of a
            # shared [12,197] PSUM tile.
            oneblock = pp.tile([128, NH * NH], BF, name="oneblock", tag="oneblock")
            nc.vector.memset(oneblock[:, :], 0.0)
            for h in range(NH):
                nc.vector.memset(oneblock[:, h * NH + h:h * NH + h + 1], 1.0)

            # ---- phase 1: load x, cast bf16 (ACT), PE-transpose into xT
            # x staging shares slots with the weight staging (tag "wstg") so
            # no pool-release barrier separates the x pipeline from weight
            # DMAs — they stream through the same 4 slots back to back.
            with tc.tile_pool(name="pst", bufs=4, space="PSUM") as pst:
                for mt, (mb, msz) in enumerate(MTILES):
                    xn = sp.tile([128, C], FP, bufs=4, name=f"xn{mt}", tag="wstg")
                    nc.sync.dma_start(xn[0:msz, :], xf[mb:mb + msz, :])
                    xb = sp.tile([128, C], BF, bufs=3, name=f"xb{mt}", tag="xb")
                    nc.vector.tensor_copy(xb[0:msz, :], xn[0:msz, :])
                    for ct in range(6):
                        pt = pst.tile([128, 128], BF, name=f"pt{mt}_{ct}", tag="pt")
                        nc.tensor.transpose(pt[:, 0:msz],
                                            xb[0:msz, ct * 128:(ct + 1) * 128],
                                            ident[0:msz, 0:msz])
                        nc.scalar.activation(xT[ct][:, mb:mb + msz], pt[:, 0:msz],
                                             AT.Copy)

            # ---- weights: chunked DMA fp32 staging -> bf16 resident (DVE cast)
            def load_cast(name, p, fdim, src_ap):
                dst = pp.tile([p, fdim], BF, name=name, tag=name)
                for cb in range(0, fdim, C):
                    csz = min(C, fdim - cb)
                    stg = sp.tile([128, C], FP, bufs=4,
                                  name=f"stg_{name}_{cb}", tag="wstg")
                    nc.sync.dma_start(stg[0:p, 0:csz], src_ap[:, cb:cb + csz])
                    nc.vector.tensor_copy(dst[:, cb:cb + csz], stg[0:p, 0:csz])
                return dst

            def load_col(name, p, src_ap):
                t = pp.tile([p, 1], FP, name=name, tag=name)
                nc.sync.dma_start(t[:, :], src_ap[:, None])
                return t

            def load_bcast(name, src_ap):
                t = pp.tile([128, C], FP, name=name, tag=name)
                nc.sync.dma_start(t[:, :], src_ap[None, :].broadcast_to((128, C)))
                return t

            aw1 = [load_cast(f"aw1_{k}", 128, HADP,
                             aw1_e[k * 128:(k + 1) * 128, :]) for k in range(6)]
            aw2 = [load_cast("aw2_0", 128, C, aw2_e[0:128, :]),
                   load_cast("aw2_1", 64, C, aw2_e[128:HADP, :])]
            qkvw = [load_cast(f"qkvw{k}", 128, 3 * C,
                              qkvw_e[k * 128:(k + 1) * 128, :]) for k in range(6)]
            projw = [load_cast(f"projw{k}", 128, C,
                               projw_e[k * 128:(k + 1) * 128, :]) for k in range(6)]

            b1c = [load_col("b1c0", 128, ab1_e[0:128]),
                   load_col("b1c1", 64, ab1_e[128:HADP])]
            b2c = [load_col(f"b2c{i}", 128, ab2_e[i * 128:(i + 1) * 128])
                   for i in range(6)]
            qkbc = [load_col(f"qkbc{i}", 128, qkvb_e[i * 128:(i + 1) * 128])
                    for i in range(12)]
            vbb = load_bcast("vbb", qkvb_e[2 * C:3 * C])
            pbb = load_bcast("pbb", projb_e[:])

            # ---- phase 2+3: adapter (hT, then x1T in place into xT)
            with tc.tile_pool(name="psA", bufs=4, space="PSUM") as psA:
                hT = [sp.tile([128, M], BF, bufs=1, name="hT0", tag="hT0"),
                      sp.tile([64, M], BF, bufs=1, name="hT1", tag="hT1")]
                for ht, (hb, hsz) in enumerate([(0, 128), (128, 64)]):
                    for mb, msz in MCHUNKS:
                        ps = psA.tile([128, 512], FP, name=f"psh{ht}_{mb}", tag="psA")
                        for kt in range(6):
                            nc.tensor.matmul(ps[0:hsz, 0:msz],
                                             aw1[kt][:, hb:hb + hsz],
                                             xT[kt][:, mb:mb + msz],
                                             start=(kt == 0), stop=(kt == 5))
                        nc.scalar.activation(hT[ht][:, mb:mb + msz], ps[0:hsz, 0:msz],
                                             AT.Identity, bias=b1c[ht][:, :])
                for ct in range(6):
                    for mb, msz in MCHUNKS:
                        ps = psA.tile([128, 512], FP, name=f"psx{ct}_{mb}", tag="psA")
                        for kt, ksz in enumerate([128, 64]):
                            nc.tensor.matmul(ps[:, 0:msz],
                                             aw2[kt][0:ksz, ct * 128:(ct + 1) * 128],
                                             hT[kt][0:ksz, mb:mb + msz],
                                             start=(kt == 0), stop=(kt == 1))
                        nc.vector.scalar_tensor_tensor(
                            out=xT[ct][:, mb:mb + msz], in0=ps[:, 0:msz],
                            scalar=b2c[ct][:, :], in1=xT[ct][:, mb:mb + msz],
                            op0=OP.add, op1=OP.add)

                # ---- phase 4a: qT / kT (channel-major)
                for ot in range(12):
                    dst = qT[ot] if ot < 6 else kT[ot - 6]
                    for mb, msz in MCHUNKS:
                        ps = psA.tile([128, 512], FP, name=f"psqk{ot}_{mb}", tag="psA")
                        for kt in range(6):
                            nc.tensor.matmul(ps[:, 0:msz],
                                             qkvw[kt][:, ot * 128:(ot + 1) * 128],
                                             xT[kt][:, mb:mb + msz],
                                             start=(kt == 0), stop=(kt == 5))
                        nc.scalar.activation(dst[:, mb:mb + msz], ps[:, 0:msz],
                                             AT.Identity, bias=qkbc[ot][:, :])

                # ---- phase 4b: v (token-major, per frame, interleaved ones)
                for f in range(F):
                    for jt, (jb, jsz) in enumerate(JTILES):
                        for half in range(2):
                            ps = psA.tile([128, 512], FP,
                                          name=f"psv{f}_{jt}_{half}", tag="psA")
                            for kt in range(6):
                                nc.tensor.matmul(
                                    ps[0:jsz, 0:384],
                                    xT[kt][:, f * N + jb: f * N + jb + jsz],
                                    qkvw[kt][:, 1536 + half * 384: 1536 + (half + 1) * 384],
                                    start=(kt == 0), stop=(kt == 5))
                            nc.vector.tensor_tensor(
                                out=vt[f][jt][0:jsz, half * 384:(half + 1) * 384],
                                in0=ps[0:jsz, 0:384],
                                in1=vbb[0:jsz, half * 384:(half + 1) * 384],
                                op=OP.add)

            # ---- phase 5: attention, per frame
            # Scores/exps as in the per-head-pair scheme (full-bank PSUM tiles,
            # row-group packing). Denominators: 24 accumulating one-hot
            # matmuls into one [12,197] bank, ONE batched DVE reciprocal per
            # frame (reciprocal cost is free-size-bound), DMA bounce through
            # DRAM to row-broadcast, then av-matmul + one DVE mult per head.
            def fk_of(f, h):
                if h < 2:
                    return max(f - 1, 0)
                if h < 4:
                    return min(f + 1, F - 1)
                return f

            with tc.tile_pool(name="psT", bufs=1, space="PSUM") as psT, \
                 tc.tile_pool(name="drp", bufs=2, space="DRAM") as drp:
                for f in range(F):
                    es = {}
                    for hp_ in range(6):
                        fk = fk_of(f, 2 * hp_)
                        for hi in range(2):
                            pb = hi * 64
                            for jt, (jb, jsz) in enumerate(JTILES):
                                ps = psT.tile([128, N], FP, bufs=5,
                                              name=f"st{f}_{hp_}_{hi}_{jt}", tag="st")
                                nc.tensor.matmul(
                                    ps[0:jsz, :],
                                    kT[hp_][pb:pb + 64, fk * N + jb: fk * N + jb + jsz],
                                    qT[hp_][pb:pb + 64, f * N:(f + 1) * N],
                                    start=True, stop=True)
                                e = sp.tile([128, N], BF, bufs=26,
                                            name=f"e{f}_{hp_}_{hi}_{jt}", tag="e")
                                nc.scalar.activation(e[0:jsz, :], ps[0:jsz, :],
                                                     AT.Exp, scale=SCALE)
                                es[(2 * hp_ + hi, jt)] = e
                    den12 = psT.tile([NH, N], FP, bufs=3,
                                     name=f"den{f}", tag="avden")
                    nmm = 0
                    for h in range(NH):
                        for jt, (jb, jsz) in enumerate(JTILES):
                            nc.tensor.matmul(
                                den12[:, :],
                                oneblock[0:jsz, h * NH:(h + 1) * NH],
                                es[(h, jt)][0:jsz, :],
                                start=(nmm == 0), stop=(nmm == 2 * NH - 1))
                            nmm += 1
                    rec12 = sp.tile([NH, N], FP, bufs=2, name=f"rcp{f}",
                                    tag="rec12")
                    nc.vector.reciprocal(rec12[:, :], den12[:, :])
                    dr12 = drp.tile([NH, N], FP, name=f"dr12_{f}", tag="dr12")
                    nc.sync.dma_start(dr12[:, :], rec12[:, :])
                    rec64s = []
                    for h in range(NH):
                        rec64 = sp.tile([64, N], FP, bufs=6,
                                        name=f"rec64_{f}_{h}", tag="rec64")
                        nc.sync.dma_start(rec64[:, :],
                                          dr12[h, :][None, :].broadcast_to((64, N)))
                        rec64s.append(rec64)
                    for h in range(NH):
                        fk = fk_of(f, h)
                        av = psT.tile([64, N], FP, bufs=3,
                                      name=f"av{f}_{h}", tag="avden")
                        for jt, (jb, jsz) in enumerate(JTILES):
                            nc.tensor.matmul(av[:, :],
                                             vt[fk][jt][0:jsz, h * HD:(h + 1) * HD],
                                             es[(h, jt)][0:jsz, :],
                                             start=(jt == 0), stop=(jt == 1))
                        nc.vector.tensor_tensor(
                            out=aoT[h // 2][(h % 2) * 64:(h % 2 + 1) * 64,
                                            f * N:(f + 1) * N],
                            in0=av[:, :],
                            in1=rec64s[h][:, :],
                            op=OP.mult)

            # ---- phase 6: proj + bias, DMA out
            with tc.tile_pool(name="psP", bufs=4, space="PSUM") as psP:
                for mt, (mb, msz) in enumerate(MTILES):
                    osb = sp.tile([128, C], FP, bufs=2, name=f"osb{mt}", tag="osb")
                    for half in range(2):
                        ps = psP.tile([128, 384], FP,
                                      name=f"psp{mt}_{half}", tag="psP")
                        for kt in range(6):
                            nc.tensor.matmul(ps[0:msz, :],
                                             aoT[kt][:, mb:mb + msz],
                                             projw[kt][:, half * 384:(half + 1) * 384],
                                             start=(kt == 0), stop=(kt == 5))
                        nc.vector.tensor_tensor(
                            out=osb[0:msz, half * 384:(half + 1) * 384],
                            in0=ps[0:msz, :],
                            in1=pbb[0:msz, half * 384:(half + 1) * 384],
                            op=OP.add)
                    nc.sync.dma_start(outf[mb:mb + msz, :], osb[0:msz, :])

    nc.compile()
    return nc


def _get_nc():
    if "nc" not in _CACHE:
        _CACHE["nc"] = _build()
    return _CACHE["nc"]


def _in_maps(inputs):
    x = np.ascontiguousarray(np.asarray(inputs["x"], np.float32))
    w = {k: np.ascontiguousarray(np.asarray(inputs[k], np.float32))
         for k in ("a_w1", "a_b1", "a_w2", "a_b2", "qkv_w", "qkv_b",
                   "proj_w", "proj_b")}
    maps = []
    for i in range(NCORES):
        m = {"x": x[i * F:(i + 1) * F]}
        m.update(w)
        maps.append(m)
    return maps


def kernel(**inputs):
    from concourse.bass_utils import run_bass_kernel_spmd
    nc = _get_nc()
    res = run_bass_kernel_spmd(nc, _in_maps(inputs), core_ids=list(range(NCORES)))
    return np.concatenate([res.results[i]["out"] for i in range(NCORES)], axis=0)


def run_traced(inputs, **kwargs):
    """Test harness helper: run with NTFF profiling, return (output, results)."""
    from concourse.bass_utils import run_bass_kernel_spmd
    nc = _get_nc()
    res = run_bass_kernel_spmd(nc, _in_maps(inputs),
                               core_ids=list(range(NCORES)), trace=True, **kwargs)
    out = np.concatenate([res.results[i]["out"] for i in range(NCORES)], axis=0)
    return out, res
